# revision 1
# baseline (speedup 1.0000x reference)
"""Trainium2 Bass kernel for nn_FreqCrossAttention.

Sharding: 8 cores = 4 batches x 2 head-groups (8 heads each).
Each core computes a partial output [2048, 1024] (its head-group's
contribution through W_o row-parallel); host sums the pair per batch.

Pipeline per core (all matmuls fp32r except attention-weights/V in bf16):
  LN(q) -> qn                         (stats on DVE, apply on ACT)
  DFT via matmul with cos/sin matrices, F padded to 1026
  QKV projections in feature-major (transposed) layout
  scoresT = Kcat^T-blocks @ Qcat  -> exp on ACT (scale=1/8, no max-sub)
  AV: expT blocks as stationary, V||ones as moving -> out + sumexp
  normalize, iDFT via matmul, W_o partial
"""
import math
import numpy as np
import ml_dtypes

MM_BF16 = True   # matmul-operand dtype: True -> bfloat16, False -> float32r

B, L, E, H = 4, 2048, 1024, 16
D = E // H            # 64
Lf = L // 2 + 1       # 1025
FP = 1026             # padded frequency dim
NH = 8                # heads per core
P = 128
FCH = [(0, 384), (384, 384), (768, 258)]   # F chunks (all >=256, 128-aligned starts)
# F-dim partition tiles for l/output dims (incl. pad row 1025): 8 x 128 + 1 x 2
FTI = [(i * P, P) for i in range(8)] + [(1024, 2)]
# m-dim tiles (keys/values: real frequencies only, no pad row): 8 x 128 + 1 x 1
MTI = [(i * P, P) for i in range(8)] + [(1024, 1)]
ET = 8                # e-chunks of E
LT = 16               # L tiles
EPS = 1e-5

_CACHE = {}


def _dft_consts():
    f = np.arange(FP)
    t = np.arange(L)
    ang = 2.0 * np.pi * np.outer(t, f) / L            # [L, FP]
    s = 1.0 / math.sqrt(L)
    FcT = (np.cos(ang) * s).astype(np.float32)        # rhs for rfft [L, FP]
    FsT = (-np.sin(ang) * s).astype(np.float32)
    FcT[:, Lf:] = 0.0
    FsT[:, Lf:] = 0.0
    cw = np.where((f == 0) | (f == L // 2), 1.0, 2.0)[:, None]
    GcT = (cw * np.cos(ang.T) * s).astype(np.float32)  # [FP, L]
    GsT = (-cw * np.sin(ang.T) * s).astype(np.float32)
    GcT[Lf:, :] = 0.0
    GsT[Lf:, :] = 0.0
    return FcT, FsT, GcT, GsT


def _build():
    import concourse.bass as bass
    import concourse.bacc as bacc
    import concourse.mybir as mybir
    import concourse.tile as tile

    R = mybir.dt.bfloat16 if MM_BF16 else mybir.dt.float32r
    F32 = mybir.dt.float32
    BF16 = mybir.dt.bfloat16
    AF = mybir.ActivationFunctionType

    nc = bacc.Bacc("TRN2", debug=False, num_devices=8)

    q_d = nc.dram_tensor("q", [L, E], F32, kind="ExternalInput")
    kv_d = nc.dram_tensor("kv", [L, E], R, kind="ExternalInput")
    gamma_d = nc.dram_tensor("gamma", [E, 1], F32, kind="ExternalInput")
    beta_d = nc.dram_tensor("beta", [E, 1], F32, kind="ExternalInput")
    FcT_d = nc.dram_tensor("FcT", [L, FP], R, kind="ExternalInput")
    FsT_d = nc.dram_tensor("FsT", [L, FP], R, kind="ExternalInput")
    GcT_d = nc.dram_tensor("GcT", [FP, L], R, kind="ExternalInput")
    GsT_d = nc.dram_tensor("GsT", [FP, L], R, kind="ExternalInput")
    W_d = {}
    for nm in ("qr", "qi", "kr", "ki", "vr", "vi"):
        W_d[nm] = nc.dram_tensor(f"W{nm}", [E, 512], R, kind="ExternalInput")
        W_d["b" + nm] = nc.dram_tensor(f"b{nm}", [512, 1], F32, kind="ExternalInput")
    WoT_d = nc.dram_tensor("WoT", [512, E], R, kind="ExternalInput")
    out_d = nc.dram_tensor("out", [L, E], F32, kind="ExternalOutput")

    with tile.TileContext(nc) as tc:
        with tc.tile_pool(name="dram", bufs=1, space="DRAM") as dram, \
             tc.tile_pool(name="persist", bufs=1) as persist:
            qn_dram = dram.tile([L, E], R)
            Qcat_dram = dram.tile([NH, P, FP], R)
            Kcat_dram = dram.tile([NH, P, FP], R)
            Vcat_dram = dram.tile([FP, NH * 129], BF16)

            # small persistent constants
            eps_t = persist.tile([P, 1], F32)
            nc.vector.memset(eps_t[:], EPS)
            gam = []
            bet = []
            for eb in range(ET):
                g = persist.tile([P, 1], F32, tag=f"gam{eb}", name=f"gam{eb}")
                nc.sync.dma_start(g[:], gamma_d.ap()[eb * P:(eb + 1) * P, :])
                gam.append(g)
                bt_ = persist.tile([P, 1], F32, tag=f"bet{eb}", name=f"bet{eb}")
                nc.sync.dma_start(bt_[:], beta_d.ap()[eb * P:(eb + 1) * P, :])
                bet.append(bt_)

            # ---------------- Phase LN: qn = LN(q) -> qn_dram ----------------
            with tc.tile_pool(name="ln", bufs=3) as ln, \
                 tc.tile_pool(name="lns", bufs=4) as lns:
                for lc in range(LT):
                    qt = ln.tile([P, E], F32, tag="qt", name="qt")
                    nc.sync.dma_start(qt[:], q_d.ap()[lc * P:(lc + 1) * P, :])
                    st = lns.tile([P, 12], F32, tag="st", name="st")
                    nc.vector.bn_stats(st[:, 0:6], qt[:, 0:512])
                    nc.vector.bn_stats(st[:, 6:12], qt[:, 512:1024])
                    mv = lns.tile([P, 2], F32, tag="mv", name="mv")
                    nc.vector.bn_aggr(mv[:], st[:])
                    sd = lns.tile([P, 1], F32, tag="sd", name="sd")
                    nc.scalar.activation(sd[:], mv[:, 1:2], AF.Sqrt, bias=eps_t[:])
                    istd = lns.tile([P, 1], F32, tag="istd", name="istd")
                    nc.vector.reciprocal(istd[:], sd[:])
                    nmu = lns.tile([P, 1], F32, tag="nmu", name="nmu")
                    nc.vector.tensor_scalar_mul(nmu[:], mv[:, 0:1], -1.0)
                    nc.vector.tensor_mul(nmu[:], nmu[:], istd[:])
                    qnt = ln.tile([P, E], R, tag="qnt", name="qnt")
                    nc.scalar.activation(qnt[:], qt[:], AF.Identity,
                                         bias=nmu[:], scale=istd[:])
                    nc.sync.dma_start(qn_dram[lc * P:(lc + 1) * P, :], qnt[:])

            # beta folded into DFT: qf_r[:, 0] += beta * sqrt(L)
            # gamma folded into DFT eviction as per-partition scale.

            # ------------- Phase A/B: DFT + projections, per fc chunk -------------
            def dft_proj(src_dram, is_q):
                # weights for this path
                with tc.tile_pool(name="wp", bufs=1) as wp:
                    names = ("qr", "qi") if is_q else ("kr", "ki", "vr", "vi")
                    Wt = {}
                    bias_t = {}
                    for nm in names:
                        Wt[nm] = [wp.tile([P, 512], R, tag=f"W{nm}{ec}", name=f"W{nm}{ec}") for ec in range(ET)]
                        for ec in range(ET):
                            nc.sync.dma_start(Wt[nm][ec][:], W_d[nm].ap()[ec * P:(ec + 1) * P, :])
                        if nm in ("qr", "qi", "kr", "ki"):
                            bias_t[nm] = [wp.tile([P, 1], F32, tag=f"b{nm}{mt}", name=f"b{nm}{mt}") for mt in range(4)]
                            for mt in range(4):
                                nc.sync.dma_start(bias_t[nm][mt][:],
                                                  W_d["b" + nm].ap()[mt * P:(mt + 1) * P, :])
                    vbias = None
                    if not is_q:
                        vb_row = wp.tile([1, 512], F32, tag="vbrow", name="vbrow")
                        vbias = {}
                        for nm in ("vr", "vi"):
                            nc.sync.dma_start(vb_row[:], W_d["b" + nm].ap().rearrange("e one -> one e"))
                            vb = wp.tile([P, 512], F32, tag=f"vb{nm}", name=f"vb{nm}")
                            nc.gpsimd.partition_broadcast(vb[:], vb_row[:])
                            vbias[nm] = vb
                        ones_row = None

                    cat_dram = Qcat_dram if is_q else Kcat_dram

                    for fci, (f0, fsz) in enumerate(FCH):
                        with tc.tile_pool(name="fslab", bufs=1) as fsl, \
                             tc.tile_pool(name="xin", bufs=3) as xin, \
                             tc.tile_pool(name="qf", bufs=1) as qfp, \
                             tc.tile_pool(name="dps", bufs=1, space="PSUM") as dps, \
                             tc.tile_pool(name="stg", bufs=4) as stg:
                            fct = fsl.tile([P, LT * fsz], R, tag="fct", name="fct")
                            fst = fsl.tile([P, LT * fsz], R, tag="fst", name="fst")
                            for lc in range(LT):
                                nc.sync.dma_start(fct[:, lc * fsz:(lc + 1) * fsz],
                                                  FcT_d.ap()[lc * P:(lc + 1) * P, f0:f0 + fsz])
                                nc.sync.dma_start(fst[:, lc * fsz:(lc + 1) * fsz],
                                                  FsT_d.ap()[lc * P:(lc + 1) * P, f0:f0 + fsz])
                            xfr = []
                            xfi = []
                            for eb in range(ET):
                                xe = xin.tile([P, LT * P], R, tag="xe", name="xe")
                                for lc in range(LT):
                                    nc.sync.dma_start(
                                        xe[:, lc * P:(lc + 1) * P],
                                        src_dram[lc * P:(lc + 1) * P, eb * P:(eb + 1) * P]
                                        if is_q else
                                        src_dram.ap()[lc * P:(lc + 1) * P, eb * P:(eb + 1) * P])
                                pr = dps.tile([P, fsz], F32, tag="pr", name="pr")
                                pi = dps.tile([P, fsz], F32, tag="pi", name="pi")
                                for lc in range(LT):
                                    nc.tensor.matmul(pr[:], xe[:, lc * P:(lc + 1) * P],
                                                     fct[:, lc * fsz:(lc + 1) * fsz],
                                                     start=(lc == 0), stop=(lc == LT - 1))
                                    nc.tensor.matmul(pi[:], xe[:, lc * P:(lc + 1) * P],
                                                     fst[:, lc * fsz:(lc + 1) * fsz],
                                                     start=(lc == 0), stop=(lc == LT - 1))
                                fr = qfp.tile([P, fsz], R, tag=f"fr{eb}", name=f"fr{eb}")
                                fi = qfp.tile([P, fsz], R, tag=f"fi{eb}", name=f"fi{eb}")
                                if is_q:
                                    nc.scalar.activation(fr[:], pr[:], AF.Identity,
                                                         scale=gam[eb][:])
                                    nc.scalar.activation(fi[:], pi[:], AF.Identity,
                                                         scale=gam[eb][:])
                                    if fci == 0:
                                        # beta contributes only to DC (f=0)
                                        nc.vector.scalar_tensor_tensor(
                                            fr[:, 0:1], bet[eb][:], math.sqrt(L),
                                            fr[:, 0:1],
                                            op0=mybir.AluOpType.mult,
                                            op1=mybir.AluOpType.add)
                                else:
                                    nc.vector.tensor_copy(fr[:], pr[:])
                                    nc.vector.tensor_copy(fi[:], pi[:])
                                xfr.append(fr)
                                xfi.append(fi)

                            # ---- Q/K projections for this fc block ----
                            pnames = ("qr", "qi") if is_q else ("kr", "ki")
                            for mt in range(4):
                                pps = {}
                                for nm in pnames:
                                    pps[nm] = dps.tile([P, fsz], F32, tag=f"pp{nm}", name=f"pp{nm}", bufs=2)
                                for ec in range(ET):
                                    src = {pnames[0]: xfr[ec], pnames[1]: xfi[ec]}
                                    for nm in pnames:
                                        nc.tensor.matmul(pps[nm][:],
                                                         Wt[nm][ec][:, mt * P:(mt + 1) * P],
                                                         src[nm][:],
                                                         start=(ec == 0), stop=(ec == ET - 1))
                                sg = {}
                                for nm in pnames:
                                    s = stg.tile([P, fsz], R, tag=f"sg{nm}", name=f"sg{nm}")
                                    nc.scalar.activation(s[:], pps[nm][:], AF.Identity,
                                                         bias=bias_t[nm][mt][:])
                                    sg[nm] = s
                                r0, i0 = pnames
                                nc.sync.dma_start(cat_dram[2 * mt, 0:64, f0:f0 + fsz], sg[r0][0:64, :])
                                nc.sync.dma_start(cat_dram[2 * mt + 1, 0:64, f0:f0 + fsz], sg[r0][64:128, :])
                                nc.sync.dma_start(cat_dram[2 * mt, 64:128, f0:f0 + fsz], sg[i0][0:64, :])
                                nc.sync.dma_start(cat_dram[2 * mt + 1, 64:128, f0:f0 + fsz], sg[i0][64:128, :])

                            # ---- V projection (kv path only): rows m in fc block ----
                            if not is_q:
                                for (m0, msz) in MTI:
                                    if not (m0 >= f0 and m0 + msz <= f0 + fsz):
                                        continue
                                    mr = m0 - f0
                                    pvr = dps.tile([P, 512], F32, tag="pvr", name="pvr")
                                    pvi = dps.tile([P, 512], F32, tag="pvi", name="pvi")
                                    for ec in range(ET):
                                        nc.tensor.matmul(pvr[0:msz, :], xfr[ec][:, mr:mr + msz],
                                                         Wt["vr"][ec][:],
                                                         start=(ec == 0), stop=(ec == ET - 1))
                                        nc.tensor.matmul(pvi[0:msz, :], xfi[ec][:, mr:mr + msz],
                                                         Wt["vi"][ec][:],
                                                         start=(ec == 0), stop=(ec == ET - 1))
                                    vc = stg.tile([P, NH * 129], BF16, tag="vc", name="vc")
                                    vco = vc[0:msz, :].rearrange("p (h c) -> p h c", h=NH)
                                    nc.vector.tensor_add(
                                        vco[:, :, 0:64],
                                        pvr[0:msz, :].rearrange("p (h c) -> p h c", h=NH),
                                        vbias["vr"][0:msz, :].rearrange("p (h c) -> p h c", h=NH))
                                    nc.vector.tensor_add(
                                        vco[:, :, 64:128],
                                        pvi[0:msz, :].rearrange("p (h c) -> p h c", h=NH),
                                        vbias["vi"][0:msz, :].rearrange("p (h c) -> p h c", h=NH))
                                    nc.vector.memset(vco[:, :, 128:129], 1.0)
                                    nc.sync.dma_start(Vcat_dram[m0:m0 + msz, :], vc[0:msz, :])

            dft_proj(qn_dram, True)
            dft_proj(kv_d, False)

            # ---------------- Phase C: attention ----------------
            oacc_ctx = tc.tile_pool(name="oacc", bufs=1)
            oacc = oacc_ctx.__enter__()
            attn_ctx = [tc.tile_pool(name="qk", bufs=1),
                        tc.tile_pool(name="vc", bufs=1),
                        tc.tile_pool(name="expp", bufs=2),
                        tc.tile_pool(name="sps", bufs=4, space="PSUM"),
                        tc.tile_pool(name="avps", bufs=3, space="PSUM"),
                        tc.tile_pool(name="nrm", bufs=4)]
            qk, vcp, expp, sps, avps, nrm = [c.__enter__() for c in attn_ctx]
            if True:
                Qc = []
                Kc = []
                for h in range(NH):
                    qt = qk.tile([P, FP], R, tag=f"Qc{h}", name=f"Qc{h}")
                    nc.sync.dma_start(qt[:], Qcat_dram[h, :, :])
                    Qc.append(qt)
                    kt = qk.tile([P, FP], R, tag=f"Kc{h}", name=f"Kc{h}")
                    nc.sync.dma_start(kt[:], Kcat_dram[h, :, :])
                    Kc.append(kt)
                Vc = []
                for ti, (m0, msz) in enumerate(MTI):
                    vt = vcp.tile([P, NH * 129], BF16, tag=f"Vc{ti}", name=f"Vc{ti}")
                    nc.sync.dma_start(vt[0:msz, :], Vcat_dram[m0:m0 + msz, :])
                    Vc.append(vt)
                our = []
                oui = []
                for ti in range(len(FTI)):
                    our.append(oacc.tile([P, 512], R, tag=f"our{ti}", name=f"our{ti}"))
                    oui.append(oacc.tile([P, 512], R, tag=f"oui{ti}", name=f"oui{ti}"))

                for h in range(NH):
                    expts = []
                    for ti, (m0, msz) in enumerate(MTI):
                        et_ = expp.tile([P, FP], BF16, tag=f"exp{ti}", name=f"exp{ti}")
                        for (f0, fsz) in FCH:
                            ps = sps.tile([P, 384], F32, tag="sc", name="sc")
                            nc.tensor.matmul(ps[0:msz, 0:fsz], Kc[h][:, m0:m0 + msz],
                                             Qc[h][:, f0:f0 + fsz], start=True, stop=True)
                            nc.scalar.activation(et_[0:msz, f0:f0 + fsz], ps[0:msz, 0:fsz],
                                                 AF.Exp, scale=float(D ** -0.5))
                        expts.append(et_)
                    for ti, (l0, lsz) in enumerate(FTI):
                        ps = avps.tile([P, 129], F32, tag="av", name="av")
                        n = len(MTI)
                        for mi, (m0, msz) in enumerate(MTI):
                            nc.tensor.matmul(ps[0:lsz, :], expts[mi][0:msz, l0:l0 + lsz],
                                             Vc[mi][0:msz, h * 129:(h + 1) * 129],
                                             start=(mi == 0), stop=(mi == n - 1))
                        rcp = nrm.tile([P, 1], F32, tag="rcp", name="rcp")
                        nc.vector.reciprocal(rcp[0:lsz, :], ps[0:lsz, 128:129])
                        nc.vector.tensor_scalar_mul(our[ti][0:lsz, h * 64:(h + 1) * 64],
                                                    ps[0:lsz, 0:64], rcp[0:lsz, :])
                        nc.vector.tensor_scalar_mul(oui[ti][0:lsz, h * 64:(h + 1) * 64],
                                                    ps[0:lsz, 64:128], rcp[0:lsz, :])

                # ---------------- Phase D: iDFT + Wo ----------------
                for c in reversed(attn_ctx):
                    c.__exit__(None, None, None)
                with tc.tile_pool(name="gsl", bufs=1) as gsl, \
                     tc.tile_pool(name="ott", bufs=1) as ottp, \
                     tc.tile_pool(name="wop", bufs=1) as wop, \
                     tc.tile_pool(name="ost", bufs=3) as ost:
                    OTT = [ottp.tile([P, L], R, tag=f"OTT{i}", name=f"OTT{i}") for i in range(4)]
                    for half in range(2):
                        t0 = half * 1024
                        ops = ctx_d = tc.tile_pool(name=f"ops{half}", bufs=1, space="PSUM")
                        ops = ops.__enter__()
                        pst = [[ops.tile([P, 512], F32, tag=f"ph{e4}_{t2}", name=f"ph{e4}_{t2}")
                                for t2 in range(2)] for e4 in range(4)]
                        for mi, (m0, msz) in enumerate(FTI):
                            gc = gsl.tile([P, 1024], R, tag="gc", name="gc")
                            gs = gsl.tile([P, 1024], R, tag="gs", name="gs")
                            nc.sync.dma_start(gc[0:msz, :], GcT_d.ap()[m0:m0 + msz, t0:t0 + 1024])
                            nc.sync.dma_start(gs[0:msz, :], GsT_d.ap()[m0:m0 + msz, t0:t0 + 1024])
                            n = len(FTI)
                            for e4 in range(4):
                                for t2 in range(2):
                                    nc.tensor.matmul(pst[e4][t2][:],
                                                     our[mi][0:msz, e4 * P:(e4 + 1) * P],
                                                     gc[0:msz, t2 * 512:(t2 + 1) * 512],
                                                     start=(mi == 0), stop=False)
                                    nc.tensor.matmul(pst[e4][t2][:],
                                                     oui[mi][0:msz, e4 * P:(e4 + 1) * P],
                                                     gs[0:msz, t2 * 512:(t2 + 1) * 512],
                                                     start=False, stop=(mi == n - 1))
                        for e4 in range(4):
                            for t2 in range(2):
                                nc.vector.tensor_copy(
                                    OTT[e4][:, t0 + t2 * 512:t0 + (t2 + 1) * 512],
                                    pst[e4][t2][:])
                        ctx_d.__exit__(None, None, None)
                    WoT_t = [wop.tile([P, E], R, tag=f"wo{i}", name=f"wo{i}") for i in range(4)]
                    for ec in range(4):
                        nc.sync.dma_start(WoT_t[ec][:], WoT_d.ap()[ec * P:(ec + 1) * P, :])
                    wops_ctx = tc.tile_pool(name="wops", bufs=2, space="PSUM")
                    wops = wops_ctx.__enter__()
                    for tb in range(LT):
                        pso = [wops.tile([P, 512], F32, tag=f"po{eo}", name=f"po{eo}") for eo in range(2)]
                        for eo in range(2):
                            for ec in range(4):
                                nc.tensor.matmul(pso[eo][:],
                                                 OTT[ec][:, tb * P:(tb + 1) * P],
                                                 WoT_t[ec][:, eo * 512:(eo + 1) * 512],
                                                 start=(ec == 0), stop=(ec == 3))
                        ot_ = ost.tile([P, E], F32, tag="ot", name="ot")
                        for eo in range(2):
                            nc.vector.tensor_copy(ot_[:, eo * 512:(eo + 1) * 512], pso[eo][:])
                        nc.sync.dma_start(out_d.ap()[tb * P:(tb + 1) * P, :], ot_[:])
                    wops_ctx.__exit__(None, None, None)
                oacc_ctx.__exit__(None, None, None)

    nc.finalize()
    return nc


def kernel(**inputs):
    from concourse.bass_utils import run_bass_kernel_spmd

    if "nc" not in _CACHE:
        _CACHE["nc"] = _build()
        _CACHE["consts"] = _dft_consts()
    nc = _CACHE["nc"]
    FcT, FsT, GcT, GsT = _CACHE["consts"]

    rdt = ml_dtypes.bfloat16 if MM_BF16 else np.float32
    q = np.ascontiguousarray(inputs["query"], dtype=np.float32)
    kv = np.ascontiguousarray(inputs["key_value"], dtype=rdt)
    in_maps = []
    for core in range(8):
        b = core // 2
        hg = core % 2
        cs = slice(hg * 512, (hg + 1) * 512)
        m = {
            "q": q[b],
            "kv": np.ascontiguousarray(kv[b]),
            "gamma": np.ascontiguousarray(inputs["gamma"], np.float32).reshape(E, 1),
            "beta": np.ascontiguousarray(inputs["beta"], np.float32).reshape(E, 1),
            "FcT": FcT.astype(rdt), "FsT": FsT.astype(rdt),
            "GcT": GcT.astype(rdt), "GsT": GsT.astype(rdt),
            "WoT": np.ascontiguousarray(inputs["Wo"][:, cs].T.astype(rdt)),
        }
        for nm in ("qr", "qi", "kr", "ki", "vr", "vi"):
            m[f"W{nm}"] = np.ascontiguousarray(inputs["W" + nm][cs, :].T.astype(rdt))
            m[f"b{nm}"] = np.ascontiguousarray(inputs["b" + nm][cs], np.float32).reshape(512, 1)
        in_maps.append(m)

    res = run_bass_kernel_spmd(nc, in_maps, core_ids=list(range(8)))
    _CACHE["last"] = res
    out = np.empty((B, L, E), np.float32)
    for b in range(B):
        out[b] = res.results[2 * b]["out"] + res.results[2 * b + 1]["out"]
    return out



# revision 3
# speedup vs baseline: 1.6643x; 1.6643x over previous
"""Trainium2 Bass kernel for nn_FreqCrossAttention.

Sharding: 8 cores = 4 batches x 2 head-groups (8 heads each).
Each core computes a partial output [2048, 1024] (its head-group's
contribution through W_o row-parallel); host sums the pair per batch.

Pipeline per core (matmul operands bf16):
  kv path emitted first so the PE starts immediately; LN(q) runs
  concurrently on DVE/ACT.  DFT via matmul with cos/sin matrices
  (F padded to 1026), QKV projections in feature-major layout,
  Q/K/V assembled directly in SBUF (no DRAM bounce),
  scoresT = Kcat^T-blocks @ Qcat -> exp on ACT (scale=1/8, no max-sub)
  AV: expT blocks stationary, V||ones moving -> out + sumexp
  normalize, iDFT via matmul, W_o partial.
"""
import math
import numpy as np
import ml_dtypes

MM_BF16 = True

B, L, E, H = 4, 2048, 1024, 16
D = E // H            # 64
Lf = L // 2 + 1       # 1025
FP = 1026             # padded frequency dim
NH = 8                # heads per core
P = 128
FCH = [(0, 384), (384, 384), (768, 258)]   # F chunks
FTI = [(i * P, P) for i in range(8)] + [(1024, 2)]
MTI = [(i * P, P) for i in range(8)] + [(1024, 1)]
ET = 8                # e-chunks of E
LT = 16               # L tiles
EPS = 1e-5

_CACHE = {}


def _dft_consts():
    f = np.arange(FP)
    t = np.arange(L)
    ang = 2.0 * np.pi * np.outer(t, f) / L            # [L, FP]
    s = 1.0 / math.sqrt(L)
    FcT = (np.cos(ang) * s).astype(np.float32)        # rhs for rfft [L, FP]
    FsT = (-np.sin(ang) * s).astype(np.float32)
    FcT[:, Lf:] = 0.0
    FsT[:, Lf:] = 0.0
    cw = np.where((f == 0) | (f == L // 2), 1.0, 2.0)[:, None]
    GcT = (cw * np.cos(ang.T) * s).astype(np.float32)  # [FP, L]
    GsT = (-cw * np.sin(ang.T) * s).astype(np.float32)
    GcT[Lf:, :] = 0.0
    GsT[Lf:, :] = 0.0
    return FcT, FsT, GcT, GsT


def _build():
    import concourse.bass as bass
    import concourse.bacc as bacc
    import concourse.mybir as mybir
    import concourse.tile as tile

    R = mybir.dt.bfloat16 if MM_BF16 else mybir.dt.float32r
    F32 = mybir.dt.float32
    BF16 = mybir.dt.bfloat16
    AF = mybir.ActivationFunctionType

    nc = bacc.Bacc("TRN2", debug=False, num_devices=8)

    q_d = nc.dram_tensor("q", [L, E], F32, kind="ExternalInput")
    kv_d = nc.dram_tensor("kv", [L, E], R, kind="ExternalInput")
    gamma_d = nc.dram_tensor("gamma", [E, 1], F32, kind="ExternalInput")
    beta_d = nc.dram_tensor("beta", [E, 1], F32, kind="ExternalInput")
    FcT_d = nc.dram_tensor("FcT", [L, FP], R, kind="ExternalInput")
    FsT_d = nc.dram_tensor("FsT", [L, FP], R, kind="ExternalInput")
    GcT_d = nc.dram_tensor("GcT", [FP, L], R, kind="ExternalInput")
    GsT_d = nc.dram_tensor("GsT", [FP, L], R, kind="ExternalInput")
    W_d = {}
    for nm in ("qr", "qi", "kr", "ki", "vr", "vi"):
        W_d[nm] = nc.dram_tensor(f"W{nm}", [E, 512], R, kind="ExternalInput")
        W_d["b" + nm] = nc.dram_tensor(f"b{nm}", [512, 1], F32, kind="ExternalInput")
    WoT_d = nc.dram_tensor("WoT", [512, E], R, kind="ExternalInput")
    out_d = nc.dram_tensor("out", [L, E], F32, kind="ExternalOutput")

    with tile.TileContext(nc) as tc:
        with tc.tile_pool(name="dram", bufs=1, space="DRAM") as dram, \
             tc.tile_pool(name="persist", bufs=1) as persist, \
             tc.tile_pool(name="qkv", bufs=1) as qkv:
            qn_dram = dram.tile([L, E], R)

            # persistent SBUF destinations for attention operands
            Qc = [qkv.tile([P, FP], R, tag=f"Qc{h}", name=f"Qc{h}") for h in range(NH)]
            Kc = [qkv.tile([P, FP], R, tag=f"Kc{h}", name=f"Kc{h}") for h in range(NH)]
            Vc = [qkv.tile([P, NH * 129], BF16, tag=f"Vc{t}", name=f"Vc{t}")
                  for t in range(len(MTI))]

            # small persistent constants
            eps_t = persist.tile([P, 1], F32)
            nc.vector.memset(eps_t[:], EPS)
            gam = []
            bet = []
            for eb in range(ET):
                g = persist.tile([P, 1], F32, tag=f"gam{eb}", name=f"gam{eb}")
                nc.sync.dma_start(g[:], gamma_d.ap()[eb * P:(eb + 1) * P, :])
                gam.append(g)
                bt_ = persist.tile([P, 1], F32, tag=f"bet{eb}", name=f"bet{eb}")
                nc.sync.dma_start(bt_[:], beta_d.ap()[eb * P:(eb + 1) * P, :])
                bet.append(bt_)

            # ---------------- Phase LN: qn = LN(q) -> qn_dram ----------------
            # Emitted first: runs on DVE/ACT while the PE chews the kv path.
            with tc.tile_pool(name="ln", bufs=3) as ln, \
                 tc.tile_pool(name="lns", bufs=4) as lns:
                for lc in range(LT):
                    qt = ln.tile([P, E], F32, tag="qt", name="qt")
                    nc.sync.dma_start(qt[:], q_d.ap()[lc * P:(lc + 1) * P, :])
                    st = lns.tile([P, 12], F32, tag="st", name="st")
                    nc.vector.bn_stats(st[:, 0:6], qt[:, 0:512])
                    nc.vector.bn_stats(st[:, 6:12], qt[:, 512:1024])
                    mv = lns.tile([P, 2], F32, tag="mv", name="mv")
                    nc.vector.bn_aggr(mv[:], st[:])
                    sd = lns.tile([P, 1], F32, tag="sd", name="sd")
                    nc.scalar.activation(sd[:], mv[:, 1:2], AF.Sqrt, bias=eps_t[:])
                    istd = lns.tile([P, 1], F32, tag="istd", name="istd")
                    nc.vector.reciprocal(istd[:], sd[:])
                    nmu = lns.tile([P, 1], F32, tag="nmu", name="nmu")
                    nc.vector.tensor_scalar_mul(nmu[:], mv[:, 0:1], -1.0)
                    nc.vector.tensor_mul(nmu[:], nmu[:], istd[:])
                    qnt = ln.tile([P, E], R, tag="qnt", name="qnt")
                    nc.scalar.activation(qnt[:], qt[:], AF.Identity,
                                         bias=nmu[:], scale=istd[:])
                    nc.sync.dma_start(qn_dram[lc * P:(lc + 1) * P, :], qnt[:])

            # beta folded into DFT: qf_r[:, 0] += beta * sqrt(L)
            # gamma folded into DFT eviction as per-partition scale.

            # ------------- DFT + projections, kv path then q path -------------
            with tc.tile_pool(name="xep", bufs=1) as xep, \
                 tc.tile_pool(name="slab", bufs=2) as slab, \
                 tc.tile_pool(name="qf", bufs=1) as qfp, \
                 tc.tile_pool(name="dps", bufs=2, space="PSUM") as dps, \
                 tc.tile_pool(name="stg", bufs=4) as stg:

                def dft_proj(src_dram, is_q):
                    # x resident in SBUF: [l-part, (lc, e)]
                    xe = xep.tile([P, LT * E], R, tag="xe", name="xe")
                    for lc in range(LT):
                        nc.sync.dma_start(
                            xe[:, lc * E:(lc + 1) * E],
                            src_dram[lc * P:(lc + 1) * P, :] if is_q
                            else src_dram.ap()[lc * P:(lc + 1) * P, :])

                    # weights for this path
                    with tc.tile_pool(name="wp", bufs=1) as wp:
                        names = ("qr", "qi") if is_q else ("kr", "ki", "vr", "vi")
                        Wt = {}
                        bias_t = {}
                        for nm in names:
                            Wt[nm] = [wp.tile([P, 512], R, tag=f"W{nm}{ec}", name=f"W{nm}{ec}") for ec in range(ET)]
                            for ec in range(ET):
                                nc.sync.dma_start(Wt[nm][ec][:], W_d[nm].ap()[ec * P:(ec + 1) * P, :])
                            if nm in ("qr", "qi", "kr", "ki"):
                                bias_t[nm] = [wp.tile([P, 1], F32, tag=f"b{nm}{mt}", name=f"b{nm}{mt}") for mt in range(4)]
                                for mt in range(4):
                                    nc.sync.dma_start(bias_t[nm][mt][:],
                                                      W_d["b" + nm].ap()[mt * P:(mt + 1) * P, :])
                        vbias = None
                        if not is_q:
                            vb_row = wp.tile([1, 512], F32, tag="vbrow", name="vbrow")
                            vbias = {}
                            for nm in ("vr", "vi"):
                                nc.sync.dma_start(vb_row[:], W_d["b" + nm].ap().rearrange("e one -> one e"))
                                vb = wp.tile([P, 512], F32, tag=f"vb{nm}", name=f"vb{nm}")
                                nc.gpsimd.partition_broadcast(vb[:], vb_row[:])
                                vbias[nm] = vb

                        cat = Qc if is_q else Kc

                        for fci, (f0, fsz) in enumerate(FCH):
                            fct = slab.tile([P, LT * 384], R, tag="fct", name="fct")
                            fst = slab.tile([P, LT * 384], R, tag="fst", name="fst")
                            for lc in range(LT):
                                nc.sync.dma_start(fct[:, lc * fsz:(lc + 1) * fsz],
                                                  FcT_d.ap()[lc * P:(lc + 1) * P, f0:f0 + fsz])
                                nc.sync.dma_start(fst[:, lc * fsz:(lc + 1) * fsz],
                                                  FsT_d.ap()[lc * P:(lc + 1) * P, f0:f0 + fsz])
                            xfr = []
                            xfi = []
                            for eb in range(ET):
                                pr = dps.tile([P, 512], F32, tag="A", name="pA")
                                pi = dps.tile([P, 512], F32, tag="B", name="pB")
                                for lc in range(LT):
                                    xs = xe[:, lc * E + eb * P:lc * E + (eb + 1) * P]
                                    nc.tensor.matmul(pr[:, 0:fsz], xs,
                                                     fct[:, lc * fsz:(lc + 1) * fsz],
                                                     start=(lc == 0), stop=(lc == LT - 1))
                                    nc.tensor.matmul(pi[:, 0:fsz], xs,
                                                     fst[:, lc * fsz:(lc + 1) * fsz],
                                                     start=(lc == 0), stop=(lc == LT - 1))
                                fr = qfp.tile([P, 384], R, tag=f"fr{eb}", name=f"fr{eb}")
                                fi = qfp.tile([P, 384], R, tag=f"fi{eb}", name=f"fi{eb}")
                                if is_q:
                                    nc.scalar.activation(fr[:, 0:fsz], pr[:, 0:fsz], AF.Identity,
                                                         scale=gam[eb][:])
                                    nc.scalar.activation(fi[:, 0:fsz], pi[:, 0:fsz], AF.Identity,
                                                         scale=gam[eb][:])
                                    if fci == 0:
                                        # beta contributes only to DC (f=0)
                                        nc.vector.scalar_tensor_tensor(
                                            fr[:, 0:1], bet[eb][:], math.sqrt(L),
                                            fr[:, 0:1],
                                            op0=mybir.AluOpType.mult,
                                            op1=mybir.AluOpType.add)
                                else:
                                    nc.vector.tensor_copy(fr[:, 0:fsz], pr[:, 0:fsz])
                                    nc.vector.tensor_copy(fi[:, 0:fsz], pi[:, 0:fsz])
                                xfr.append(fr)
                                xfi.append(fi)

                            # ---- Q/K projections for this fc block ----
                            pnames = ("qr", "qi") if is_q else ("kr", "ki")
                            for mt in range(4):
                                pps = {pnames[0]: dps.tile([P, 512], F32, tag="C", name="pC"),
                                       pnames[1]: dps.tile([P, 512], F32, tag="D", name="pD")}
                                for ec in range(ET):
                                    src = {pnames[0]: xfr[ec], pnames[1]: xfi[ec]}
                                    for nm in pnames:
                                        nc.tensor.matmul(pps[nm][:, 0:fsz],
                                                         Wt[nm][ec][:, mt * P:(mt + 1) * P],
                                                         src[nm][:, 0:fsz],
                                                         start=(ec == 0), stop=(ec == ET - 1))
                                sg = {}
                                for nm in pnames:
                                    s = stg.tile([P, 384], R, tag=f"sg{nm}", name=f"sg{nm}")
                                    nc.scalar.activation(s[:, 0:fsz], pps[nm][:, 0:fsz], AF.Identity,
                                                         bias=bias_t[nm][mt][:])
                                    sg[nm] = s
                                r0, i0 = pnames
                                nc.sync.dma_start(cat[2 * mt][0:64, f0:f0 + fsz], sg[r0][0:64, 0:fsz])
                                nc.sync.dma_start(cat[2 * mt + 1][0:64, f0:f0 + fsz], sg[r0][64:128, 0:fsz])
                                nc.sync.dma_start(cat[2 * mt][64:128, f0:f0 + fsz], sg[i0][0:64, 0:fsz])
                                nc.sync.dma_start(cat[2 * mt + 1][64:128, f0:f0 + fsz], sg[i0][64:128, 0:fsz])

                            # ---- V projection (kv path only): rows m in fc block ----
                            if not is_q:
                                for ti, (m0, msz) in enumerate(MTI):
                                    if not (m0 >= f0 and m0 + msz <= f0 + fsz):
                                        continue
                                    mr = m0 - f0
                                    pvr = dps.tile([P, 512], F32, tag="A", name="pA")
                                    pvi = dps.tile([P, 512], F32, tag="B", name="pB")
                                    for ec in range(ET):
                                        nc.tensor.matmul(pvr[0:msz, :], xfr[ec][:, mr:mr + msz],
                                                         Wt["vr"][ec][:],
                                                         start=(ec == 0), stop=(ec == ET - 1))
                                        nc.tensor.matmul(pvi[0:msz, :], xfi[ec][:, mr:mr + msz],
                                                         Wt["vi"][ec][:],
                                                         start=(ec == 0), stop=(ec == ET - 1))
                                    vco = Vc[ti][0:msz, :].rearrange("p (h c) -> p h c", h=NH)
                                    nc.vector.tensor_add(
                                        vco[:, :, 0:64],
                                        pvr[0:msz, :].rearrange("p (h c) -> p h c", h=NH),
                                        vbias["vr"][0:msz, :].rearrange("p (h c) -> p h c", h=NH))
                                    nc.vector.tensor_add(
                                        vco[:, :, 64:128],
                                        pvi[0:msz, :].rearrange("p (h c) -> p h c", h=NH),
                                        vbias["vi"][0:msz, :].rearrange("p (h c) -> p h c", h=NH))
                                    nc.vector.memset(vco[:, :, 128:129], 1.0)

                dft_proj(kv_d, False)
                dft_proj(qn_dram, True)

            # ---------------- Phase C: attention ----------------
            oacc_ctx = tc.tile_pool(name="oacc", bufs=1)
            oacc = oacc_ctx.__enter__()
            attn_ctx = [tc.tile_pool(name="expp", bufs=2),
                        tc.tile_pool(name="sps", bufs=4, space="PSUM"),
                        tc.tile_pool(name="avps", bufs=3, space="PSUM"),
                        tc.tile_pool(name="nrm", bufs=4)]
            expp, sps, avps, nrm = [c.__enter__() for c in attn_ctx]
            if True:
                our = []
                oui = []
                for ti in range(len(FTI)):
                    our.append(oacc.tile([P, 512], R, tag=f"our{ti}", name=f"our{ti}"))
                    oui.append(oacc.tile([P, 512], R, tag=f"oui{ti}", name=f"oui{ti}"))

                for h in range(NH):
                    expts = []
                    for ti, (m0, msz) in enumerate(MTI):
                        et_ = expp.tile([P, FP], BF16, tag=f"exp{ti}", name=f"exp{ti}")
                        for (f0, fsz) in FCH:
                            ps = sps.tile([P, 384], F32, tag="sc", name="sc")
                            nc.tensor.matmul(ps[0:msz, 0:fsz], Kc[h][:, m0:m0 + msz],
                                             Qc[h][:, f0:f0 + fsz], start=True, stop=True)
                            nc.scalar.activation(et_[0:msz, f0:f0 + fsz], ps[0:msz, 0:fsz],
                                                 AF.Exp, scale=float(D ** -0.5))
                        expts.append(et_)
                    for ti, (l0, lsz) in enumerate(FTI):
                        ps = avps.tile([P, 129], F32, tag="av", name="av")
                        n = len(MTI)
                        for mi, (m0, msz) in enumerate(MTI):
                            nc.tensor.matmul(ps[0:lsz, :], expts[mi][0:msz, l0:l0 + lsz],
                                             Vc[mi][0:msz, h * 129:(h + 1) * 129],
                                             start=(mi == 0), stop=(mi == n - 1))
                        rcp = nrm.tile([P, 1], F32, tag="rcp", name="rcp")
                        nc.vector.reciprocal(rcp[0:lsz, :], ps[0:lsz, 128:129])
                        nc.vector.tensor_scalar_mul(our[ti][0:lsz, h * 64:(h + 1) * 64],
                                                    ps[0:lsz, 0:64], rcp[0:lsz, :])
                        nc.vector.tensor_scalar_mul(oui[ti][0:lsz, h * 64:(h + 1) * 64],
                                                    ps[0:lsz, 64:128], rcp[0:lsz, :])

                # ---------------- Phase D: iDFT + Wo ----------------
                for c in reversed(attn_ctx):
                    c.__exit__(None, None, None)
                with tc.tile_pool(name="gsl", bufs=2) as gsl, \
                     tc.tile_pool(name="ott", bufs=1) as ottp, \
                     tc.tile_pool(name="wop", bufs=1) as wop, \
                     tc.tile_pool(name="ost", bufs=3) as ost:
                    OTT = [ottp.tile([P, L], R, tag=f"OTT{i}", name=f"OTT{i}") for i in range(4)]
                    for half in range(2):
                        t0 = half * 1024
                        ops_ctx = tc.tile_pool(name=f"ops{half}", bufs=1, space="PSUM")
                        ops = ops_ctx.__enter__()
                        pst = [[ops.tile([P, 512], F32, tag=f"ph{e4}_{t2}", name=f"ph{e4}_{t2}")
                                for t2 in range(2)] for e4 in range(4)]
                        for mi, (m0, msz) in enumerate(FTI):
                            gc = gsl.tile([P, 1024], R, tag="gc", name="gc")
                            gs = gsl.tile([P, 1024], R, tag="gs", name="gs")
                            nc.sync.dma_start(gc[0:msz, :], GcT_d.ap()[m0:m0 + msz, t0:t0 + 1024])
                            nc.sync.dma_start(gs[0:msz, :], GsT_d.ap()[m0:m0 + msz, t0:t0 + 1024])
                            n = len(FTI)
                            for e4 in range(4):
                                for t2 in range(2):
                                    nc.tensor.matmul(pst[e4][t2][:],
                                                     our[mi][0:msz, e4 * P:(e4 + 1) * P],
                                                     gc[0:msz, t2 * 512:(t2 + 1) * 512],
                                                     start=(mi == 0), stop=False)
                                    nc.tensor.matmul(pst[e4][t2][:],
                                                     oui[mi][0:msz, e4 * P:(e4 + 1) * P],
                                                     gs[0:msz, t2 * 512:(t2 + 1) * 512],
                                                     start=False, stop=(mi == n - 1))
                        for e4 in range(4):
                            for t2 in range(2):
                                nc.vector.tensor_copy(
                                    OTT[e4][:, t0 + t2 * 512:t0 + (t2 + 1) * 512],
                                    pst[e4][t2][:])
                        ops_ctx.__exit__(None, None, None)
                    WoT_t = [wop.tile([P, E], R, tag=f"wo{i}", name=f"wo{i}") for i in range(4)]
                    for ec in range(4):
                        nc.sync.dma_start(WoT_t[ec][:], WoT_d.ap()[ec * P:(ec + 1) * P, :])
                    wops_ctx = tc.tile_pool(name="wops", bufs=2, space="PSUM")
                    wops = wops_ctx.__enter__()
                    for tb in range(LT):
                        pso = [wops.tile([P, 512], F32, tag=f"po{eo}", name=f"po{eo}") for eo in range(2)]
                        for eo in range(2):
                            for ec in range(4):
                                nc.tensor.matmul(pso[eo][:],
                                                 OTT[ec][:, tb * P:(tb + 1) * P],
                                                 WoT_t[ec][:, eo * 512:(eo + 1) * 512],
                                                 start=(ec == 0), stop=(ec == 3))
                        ot_ = ost.tile([P, E], F32, tag="ot", name="ot")
                        for eo in range(2):
                            nc.vector.tensor_copy(ot_[:, eo * 512:(eo + 1) * 512], pso[eo][:])
                        nc.sync.dma_start(out_d.ap()[tb * P:(tb + 1) * P, :], ot_[:])
                    wops_ctx.__exit__(None, None, None)
                oacc_ctx.__exit__(None, None, None)

    nc.finalize()
    return nc


def kernel(**inputs):
    from concourse.bass_utils import run_bass_kernel_spmd

    if "nc" not in _CACHE:
        _CACHE["nc"] = _build()
        _CACHE["consts"] = _dft_consts()
    nc = _CACHE["nc"]
    FcT, FsT, GcT, GsT = _CACHE["consts"]

    rdt = ml_dtypes.bfloat16 if MM_BF16 else np.float32
    q = np.ascontiguousarray(inputs["query"], dtype=np.float32)
    kv = np.ascontiguousarray(inputs["key_value"], dtype=rdt)
    in_maps = []
    for core in range(8):
        b = core // 2
        hg = core % 2
        cs = slice(hg * 512, (hg + 1) * 512)
        m = {
            "q": q[b],
            "kv": np.ascontiguousarray(kv[b]),
            "gamma": np.ascontiguousarray(inputs["gamma"], np.float32).reshape(E, 1),
            "beta": np.ascontiguousarray(inputs["beta"], np.float32).reshape(E, 1),
            "FcT": FcT.astype(rdt), "FsT": FsT.astype(rdt),
            "GcT": GcT.astype(rdt), "GsT": GsT.astype(rdt),
            "WoT": np.ascontiguousarray(inputs["Wo"][:, cs].T.astype(rdt)),
        }
        for nm in ("qr", "qi", "kr", "ki", "vr", "vi"):
            m[f"W{nm}"] = np.ascontiguousarray(inputs["W" + nm][cs, :].T.astype(rdt))
            m[f"b{nm}"] = np.ascontiguousarray(inputs["b" + nm][cs], np.float32).reshape(512, 1)
        in_maps.append(m)

    res = run_bass_kernel_spmd(nc, in_maps, core_ids=list(range(8)))
    _CACHE["last"] = res
    out = np.empty((B, L, E), np.float32)
    for b in range(B):
        out[b] = res.results[2 * b]["out"] + res.results[2 * b + 1]["out"]
    return out
